# revision 34
# baseline (speedup 1.0000x reference)
"""Trainium2 Bass kernel for nn_RelativeMultiHeadAttn (TransformerXL-style
relative multi-head attention).

Sharding: data-parallel over batch — core b handles batch element b (B=8).

Per-core math (S=512, D=1024, H=16 heads, HD=64):
  q = x @ Wq ; v = x @ Wv ; k_h = x[:, h*64:(h+1)*64]
  AC_h  = (q_h + r_r_bias_h) @ k_h^T
  X_h   = (q_h + r_w_bias_h) @ pos^T                  # [S, 2S] "diagonal coords"
  BD_h[q,k] = X_h[q, S + k - q]                       # relative shift
  out_h = softmax(AC_h + BD_h) @ v_h

Design notes (v2 — transposed-score layout):
  * Scores are accumulated TRANSPOSED: S^T[k, q] = AC^T + BD^T per
    (head, k-tile) PSUM bank. AC^T is the same matmul as AC with lhsT/rhs
    swapped (both operands already live in [d-on-partitions] layout). BD
    comes back from the skew round trip in [q, k] layout and is transposed
    by fp16 identity matmuls that accumulate straight into the score bank.
    exp(S^T) then emits P^T directly — no separate P-transpose stage, no
    vector adds for BD, no accumulator reads.
  * Softmax denominators come from a ones-column appended per head to v
    (the AV matmul computes sums in its 65th output column).
  * The relative shift is a DRAM round-trip: per (head, q-tile) a
    [128, 640] fp16 band of X is written contiguously and BD is read back
    with a skewed access pattern (row stride 639 elements, column offset
    128) landing each row's shifted 512-wide window densely.
  * X band matmuls are split 320+320 (not 512+128): PSUM banks cap matmul
    N at 512 f32, and sub-256 f32 matmuls hit rate penalties.
  * The whole score path runs in fp16 (x, Wq, q+bias, pos, band): fp32
    moving operands stream the PE at half rate on hardware; fp16 keeps
    full rate and enough mantissa (logit error ~1e-2 absolute). P stays
    bf16 (unnormalized exp reaches e^25+, overflowing fp16 range).
  * wq is loaded in dt-major chunks so the first q-projection group can
    start after ~0.25 MB instead of the full 2 MB; x^T loads first.
  * Output is written fp16 and upcast to f32 on the host.
"""

import numpy as np
import ml_dtypes

import concourse.bass as bass
import concourse.mybir as mybir
import concourse.tile as tile
from concourse.bass_utils import run_bass_kernel_spmd
from concourse.vector_clock import ScopedClock

B, S, D, H = 8, 512, 1024, 16
HD = D // H          # 64
QT = S // 128        # 4 q tiles
KT = D // 128        # 8 model-dim tiles
BAND = 640           # X band width per q-tile
XS = 320             # X band matmul split (two N=320 matmuls)
POSW = 1024          # pos table width (2S)
CSKEW = 128          # uniform skew-read column offset
f32 = mybir.dt.float32
f32r = mybir.dt.float32r
bf16 = mybir.dt.bfloat16
fp16 = mybir.dt.float16


# ---------------------------------------------------------------------------
# TileContext exit-drain workaround: this snapshot attaches every outstanding
# sem wait to one SP Drain, which walrus rejects ("Too many sync wait
# commands"). Split the waits across standalone SP nops instead.
def _drain_and_barrier_split(self, tick_clock, wait_clock):
    nc = self.nc
    probe = nc.sync.nop()
    wait_clock.add_sem_waits(probe.ins, ScopedClock({None: tick_clock.global_clock}))
    si = probe.ins.sync_info
    waits = list(si.on_wait) if si is not None else []
    if si is not None and len(waits) > 1:
        si.on_wait = [waits[0]]
        for w in waits[1:]:
            extra = nc.sync.nop()
            esi = extra.ins.sync_info
            if esi is None:
                extra.ins.sync_info = mybir.SyncInfo(on_wait=[w], on_update=[])
            else:
                esi.on_wait = [w]
    nc.sync.drain()
    nc.all_engine_barrier()
    assert self.sems is not None
    popped = nc._tile_sem_poison_stack.pop()
    assert popped is self._sem_poison
    nc.clear_and_free_semaphores(list(self.sems.allocated().values()))
    nc.all_engine_barrier()


tile.TileContext._drain_and_barrier = _drain_and_barrier_split

_wsplit_counter = [0]


def _split_excess_waits(nc, max_waits=1):
    """Walrus in this container rejects instructions carrying more than one
    sem wait ("Too many sync wait commands"), but Tile's wait-assignment pass
    can attach several. Move excess waits onto fresh NoOps inserted right
    before the instruction on the same engine."""
    for f in nc.m.functions:
        for bb in f.blocks:
            new_insts = []
            changed = False
            for inst in bb.instructions:
                si = inst.sync_info
                waits = list(si.on_wait) if si is not None else []
                if len(waits) > max_waits and inst.engine != mybir.EngineType.Unassigned:
                    for w in waits[:-max_waits]:
                        _wsplit_counter[0] += 1
                        nop = mybir.InstNoOp(
                            name=f"WSPLIT-{_wsplit_counter[0]}", ins=[], outs=[]
                        )
                        nop.engine = inst.engine
                        nop.sync_info = mybir.SyncInfo(on_wait=[w], on_update=[])
                        new_insts.append(nop)
                    si.on_wait = waits[-max_waits:]
                    changed = True
                new_insts.append(inst)
            if changed:
                bb.instructions = new_insts


def _pos_embed_np():
    """RelativeSinusoidalPositionalEmbedding table slice, [2S, HD] fp32."""
    num = 1201
    half = HD // 2
    freq = np.exp(np.arange(half, dtype=np.float32) * (-np.log(10000.0) / (half - 1)))
    pos = np.arange(-((num + 1) // 2), num // 2, dtype=np.float32)
    emb = pos[:, None] * freq[None, :]
    table = np.concatenate([np.sin(emb), np.cos(emb)], axis=1).astype(np.float32)
    table[0] = 0.0
    origin_shift = num // 2 + 1
    idx = np.arange(-S, S) + origin_shift
    return table[idx]  # [1024, 64]


# Band window start (pos-table columns) per q-tile:
#   Xt[p, j] = X[128t + p, e_t + j],  j in [0, 640)
#   BD[p, k] = Xt[p, CSKEW + k - p]
_E = [384 - 128 * t for t in range(QT)]


class _St:
    pass


def _emit_body(nc, tc, pools, tensors):
    singles, pA, pS, pX, sb_small, sb_x, sb_p = pools
    (xT_d, wq_d, wv_d, posT2_d, rrb_d, rwb_d, ident_d,
     xskew_d, out_d) = tensors

    # ---- persistent SBUF loads (x^T first: the q projection needs it) ----
    xT_sb = singles.tile([128, KT, S], fp16, name="xT_sb")
    wq_sb = singles.tile([128, KT, D], fp16, name="wq_sb")
    wv_sb = singles.tile([128, KT, D], fp16, name="wv_sb")
    xT_r = xT_d.ap().rearrange("(kt p) s -> p kt s", p=128)
    wq_r = wq_d.ap().rearrange("(kt p) d -> p kt d", p=128)
    wv_r = wv_d.ap().rearrange("(kt p) d -> p kt d", p=128)
    for kt in range(KT):
        nc.sync.dma_start(out=xT_sb[:, kt], in_=xT_r[:, kt])
    # wq dt-major: q-projection group dt needs only chunk dt
    nc.sync.dma_start(out=wq_sb[:, :, 0:128], in_=wq_r[:, :, 0:128])

    posT2_sb = singles.tile([128, POSW], fp16, name="posT2_sb")
    nc.sync.dma_start(out=posT2_sb, in_=posT2_d.ap())
    rrb_sb = singles.tile([128, KT], f32, name="rrb_sb")
    nc.sync.dma_start(out=rrb_sb, in_=rrb_d.ap())
    rwb_sb = singles.tile([128, KT], f32, name="rwb_sb")
    nc.sync.dma_start(out=rwb_sb, in_=rwb_d.ap())
    ident_sb = singles.tile([128, 128], fp16, name="ident_sb")
    nc.sync.dma_start(out=ident_sb, in_=ident_d.ap())

    for dt in range(1, KT):
        nc.sync.dma_start(
            out=wq_sb[:, :, dt * 128 : (dt + 1) * 128],
            in_=wq_r[:, :, dt * 128 : (dt + 1) * 128],
        )
    for half in range(2):
        nc.sync.dma_start(
            out=wv_sb[:, :, half * 512 : (half + 1) * 512],
            in_=wv_r[:, :, half * 512 : (half + 1) * 512],
        )

    rwq_sb = singles.tile([128, KT, S], fp16, name="rwq_sb")
    rwq2_sb = singles.tile([128, KT, S], fp16, name="rwq2_sb")
    # v with a ones column per head: [128, kc, H*(HD+1)]
    v_sb = singles.tile([128, QT, H * (HD + 1)], bf16, name="v_sb")
    v_v = v_sb.rearrange("p kc (h c) -> p kc h c", c=HD + 1)
    nc.vector.memset(v_v[:, :, :, HD], 1.0)
    out_sb = singles.tile([128, QT, D], fp16, name="out_sb")

    _xcopy_ctr = [0]

    def emit_qt_group(dt):
        """q^T chunk dt = Wq^T @ x^T plus the two bias variants (DVE)."""
        q_ps = pA.tile([128, S], f32, name="q_ps", tag="pa")
        for kt in range(KT):
            nc.tensor.matmul(
                q_ps,
                lhsT=wq_sb[:, kt, dt * 128 : (dt + 1) * 128],
                rhs=xT_sb[:, kt, :],
                start=(kt == 0),
                stop=(kt == KT - 1),
            )
        nc.vector.tensor_tensor(
            out=rwq_sb[:, dt, :], in0=q_ps,
            in1=rrb_sb[:, dt : dt + 1].to_broadcast((128, S)),
            op=mybir.AluOpType.add,
        )
        nc.scalar.activation(
            out=rwq2_sb[:, dt, :], in_=q_ps,
            func=mybir.ActivationFunctionType.Identity,
            bias=rwb_sb[:, dt : dt + 1],
        )

    def emit_v_group(kc, half):
        v_ps = pA.tile([128, S], f32, name="v_ps", tag="pa")
        for kt in range(KT):
            nc.tensor.matmul(
                v_ps,
                lhsT=xT_sb[:, kt, kc * 128 : (kc + 1) * 128],
                rhs=wv_sb[:, kt, half * 512 : (half + 1) * 512],
                start=(kt == 0),
                stop=(kt == KT - 1),
            )
        nc.vector.tensor_copy(
            out=v_v[:, kc, half * 8 : (half + 1) * 8, :HD],
            in_=v_ps.rearrange("p (h c) -> p h c", c=HD),
        )

    def emit_x_tile(st, j, t):
        """X band matmuls for both heads of pair j at q-tile t, interleaved
        so the two K=64 row groups stream the PE concurrently, then one
        merged psum->sbuf fp16 copy per head."""
        pair = (2 * j, 2 * j + 1)
        e_t = _E[t]
        dt = j
        x_ps = {}
        for h in pair:
            x_ps[h] = pX.tile([128, 1024], f32, name="x_ps", tag=f"px{h % 2}")
        for u in range(2):
            for h in pair:
                qs = 64 * (h % 2)
                lq2 = rwq2_sb[qs : qs + 64, dt, t * 128 : (t + 1) * 128]
                nc.tensor.matmul(
                    x_ps[h][:, u * 512 : u * 512 + XS], lhsT=lq2,
                    rhs=posT2_sb[qs : qs + 64, e_t + u * XS : e_t + (u + 1) * XS],
                    start=True, stop=True,
                )
        for h in pair:
            src = x_ps[h].rearrange("p (u c) -> p u c", c=512)[:, :, 0:XS]
            dst = st.x_sbh[h % 2][:, t, :].rearrange("p (u c) -> p u c", c=XS)
            # all X copies on DVE: ACT's strict 8-deep FIFO would queue
            # them between exps and stall the score chain
            nc.vector.tensor_copy(out=dst, in_=src)

    def emit_x_pair(st, j):
        """X bands + skew write/read DMAs for head pair (2j, 2j+1)."""
        pair = (2 * j, 2 * j + 1)
        st.x_sbh = {}
        st.bd_sbh = {}
        for h in pair:
            st.x_sbh[h % 2] = sb_x.tile(
                [128, QT, BAND], fp16, name=f"x_sbh{h % 2}", tag=f"x_sbh{h % 2}"
            )
            st.bd_sbh[h % 2] = sb_x.tile(
                [128, QT, 512], fp16, name=f"bd_sbh{h % 2}", tag=f"bd_sbh{h % 2}"
            )
        for t in range(QT):
            emit_x_tile(st, j, t)
        for h in pair:
            nc.sync.dma_start(
                out=xskew_d.ap()[h].rearrange("t p j -> p t j"),
                in_=st.x_sbh[h % 2],
            )
            nc.sync.dma_start(
                out=st.bd_sbh[h % 2],
                in_=bass.AP(
                    xskew_d,
                    h * QT * 128 * BAND + CSKEW,
                    [[BAND - 1, 128], [128 * BAND, QT], [1, 512]],
                ),
            )

    def emit_scores_av(st, j):
        """S^T accumulate + exp -> P^T, then AV + normalize, for pair j."""
        pair = (2 * j, 2 * j + 1)
        PT_sb = {}
        recip_sb = {}
        for h in pair:
            PT_sb[h] = sb_p.tile([128, QT, S], bf16, name="PT_sb", tag=f"PT{h % 2}")
            recip_sb[h] = sb_small.tile([128, QT], f32, name="recip_sb",
                                        tag=f"recip{h % 2}")
        # scores: per kc: the two heads' AC^T back-to-back (K=64 row groups
        # stream concurrently — the shared 2-slot score tag lets both banks
        # fill at once), then the fp16 identity transposes of BD
        # accumulating into the same banks, then the two exps
        for kc in range(QT):
            s_ps = {}
            for h in pair:
                s_ps[h] = pS.tile([128, 512], f32, name="s_ps", tag="ps")
            for h in pair:
                qs = 64 * (h % 2)
                dt = h // 2
                nc.tensor.matmul(
                    s_ps[h],
                    lhsT=xT_sb[qs : qs + 64, dt, kc * 128 : (kc + 1) * 128],
                    rhs=rwq_sb[qs : qs + 64, dt, :],
                    start=True, stop=False,
                )
            for h in pair:
                for t in range(QT):
                    nc.tensor.matmul(
                        s_ps[h][:, t * 128 : (t + 1) * 128],
                        lhsT=st.bd_sbh[h % 2][:, t, kc * 128 : (kc + 1) * 128],
                        rhs=ident_sb,
                        start=False, stop=(t == QT - 1),
                    )
            for h in pair:
                nc.scalar.activation(
                    out=PT_sb[h][:, kc, :], in_=s_ps[h],
                    func=mybir.ActivationFunctionType.Exp,
                )
        # AV: out[q, d] per head, plus sums in column HD via the ones column
        for h in pair:
            av_ps = pA.tile([128, QT * (HD + 1)], f32, name="av_ps", tag="pa")
            av_v = av_ps.rearrange("p (t c) -> p t c", c=HD + 1)
            for t in range(QT):
                for kc in range(QT):
                    nc.tensor.matmul(
                        av_v[:, t, :],
                        lhsT=PT_sb[h][:, kc, t * 128 : (t + 1) * 128],
                        rhs=v_sb[:, kc, h * (HD + 1) : (h + 1) * (HD + 1)],
                        start=(kc == 0), stop=(kc == QT - 1),
                    )
            nc.vector.reciprocal(out=recip_sb[h], in_=av_v[:, :, HD])
            nc.vector.tensor_tensor(
                out=out_sb[:, :, h * HD : (h + 1) * HD],
                in0=av_v[:, :, 0:HD],
                in1=recip_sb[h][:, :, None].to_broadcast((128, QT, HD)),
                op=mybir.AluOpType.mult,
            )
        out_r = out_d.ap().rearrange("(t p) d -> p t d", p=128)
        c0 = 2 * j * HD
        nc.sync.dma_start(
            out=out_r[:, :, c0 : c0 + 2 * HD],
            in_=out_sb[:, :, c0 : c0 + 2 * HD],
        )

    # ---- schedule: X band + skew round trip runs one head-pair ahead of
    # the scores/AV consumption; q^T and v projections fill PE gaps --------
    st = _St()
    emit_qt_group(0)
    emit_x_pair(st, 0)
    emit_qt_group(1)
    for kc in range(QT):
        emit_v_group(kc, 0)
    prev = dict(st.__dict__)
    for j in range(1, H // 2):
        emit_x_pair(st, j)
        cur = dict(st.__dict__)
        if j + 1 < H // 2:
            emit_qt_group(j + 1)
        if j in (2, 3):
            for kc in (0, 1) if j == 2 else (2, 3):
                emit_v_group(kc, 1)
        stp = _St()
        stp.__dict__.update(prev)
        emit_scores_av(stp, j - 1)
        prev = cur
    stp = _St()
    stp.__dict__.update(prev)
    emit_scores_av(stp, H // 2 - 1)


def build_nc(n_repeat=1):
    nc = bass.Bass(
        trn_type="TRN2", target_bir_lowering=False, debug=False,
        num_devices=8, name="relattn",
    )
    xT_d = nc.dram_tensor("xt", [D, S], fp16, kind="ExternalInput")
    wq_d = nc.dram_tensor("wq", [D, D], fp16, kind="ExternalInput")
    wv_d = nc.dram_tensor("wv", [D, D], fp16, kind="ExternalInput")
    posT2_d = nc.dram_tensor("post2", [128, POSW], fp16, kind="ExternalInput")
    rrb_d = nc.dram_tensor("rrb", [128, KT], f32, kind="ExternalInput")
    rwb_d = nc.dram_tensor("rwb", [128, KT], f32, kind="ExternalInput")
    ident_d = nc.dram_tensor("ident", [128, 128], fp16, kind="ExternalInput")
    xskew_d = nc.dram_tensor("xskew", [H, QT, 128, BAND], fp16)
    out_d = nc.dram_tensor("out", [S, D], fp16, kind="ExternalOutput")
    tensors = (xT_d, wq_d, wv_d, posT2_d, rrb_d, rwb_d, ident_d,
               xskew_d, out_d)

    with tile.TileContext(nc) as tc:
        with (
            tc.tile_pool(name="singles", bufs=1) as singles,
            tc.tile_pool(name="pA", bufs=2, space="PSUM") as pA,
            tc.tile_pool(name="pS", bufs=2, space="PSUM") as pS,
            tc.tile_pool(name="pX", bufs=1, space="PSUM") as pX,
            tc.tile_pool(name="sb_small", bufs=2) as sb_small,
            tc.tile_pool(name="sb_x", bufs=2) as sb_x,
            tc.tile_pool(name="sb_p", bufs=2) as sb_p,
        ):
            pools = (singles, pA, pS, pX, sb_small, sb_x, sb_p)
            if n_repeat == 1:
                _emit_body(nc, tc, pools, tensors)
            else:
                with tc.For_i(0, n_repeat, 1):
                    _emit_body(nc, tc, pools, tensors)
    _split_excess_waits(nc)
    return nc


def make_in_maps(inputs):
    x = np.asarray(inputs["x"], dtype=np.float32)
    Wqv = np.asarray(inputs["Wqv"], dtype=np.float32)
    rrb = np.asarray(inputs["r_r_bias"], dtype=np.float32)
    rwb = np.asarray(inputs["r_w_bias"], dtype=np.float32)

    pos = _pos_embed_np()                       # [1024, 64]
    posT = np.ascontiguousarray(pos.T)          # [64, 1024]
    posT2 = np.concatenate([posT, posT], axis=0).astype(np.float16)
    wq = np.ascontiguousarray(Wqv[:, :D]).astype(np.float16)
    wv = np.ascontiguousarray(Wqv[:, D:]).astype(np.float16)
    rrb_col = np.ascontiguousarray(rrb.reshape(KT, 128).T)
    rwb_col = np.ascontiguousarray(rwb.reshape(KT, 128).T)

    in_maps = []
    for b in range(B):
        in_maps.append({
            "xt": np.ascontiguousarray(x[b].T).astype(np.float16),
            "wq": wq,
            "wv": wv,
            "post2": posT2,
            "rrb": rrb_col,
            "rwb": rwb_col,
            "ident": np.eye(128, dtype=np.float16),
        })
    return in_maps


_cached = {}


def run(inputs, n_repeat=1):
    if n_repeat not in _cached:
        _cached[n_repeat] = build_nc(n_repeat)
    nc = _cached[n_repeat]
    in_maps = make_in_maps(inputs)
    res = run_bass_kernel_spmd(nc, in_maps, core_ids=list(range(B)))
    out = np.stack([res.results[b]["out"] for b in range(B)], axis=0)
    return out.astype(np.float32)


def kernel(**inputs) -> np.ndarray:
    return run(inputs, n_repeat=1)
